# revision 11
# baseline (speedup 1.0000x reference)
"""Trainium2 Bass kernel for nn_AcPredict (action-conditional predict step).

Self-contained: takes full (unsharded) inputs, shards batch across 8
NeuronCores (pure data parallel), runs one SPMD Bass kernel, gathers the
full outputs.

Math (per batch element b):
    coef = softmax(post_mean @ w_coef.T + b_coef)            [15]
    t_mn = band(sum_k coef_k * tm_mn[k]) (+ I for t11/t22)   [60x60], bw=3
    next_mean = [t11 mu + t12 ml; t21 mu + t22 ml] + control_net(action)
    ncu = diag-cov predict (quadratic in t), etc.

The band mask (bandwidth 3) means every t-matrix has only 7 diagonals, so
the kernel works with diagonal slabs t[b, i, o] (o in 0..6, j = i+o-3)
instead of dense 60x60 matrices:
  - slabs come from two [15]->[840] matmuls per matrix-pair (PE, fp32)
  - softmax normalization is folded in by mixing with raw exp() weights and
    accumulating S = sum(exp) onto the identity diagonal; all outputs are
    divided by S (means) or S^2 (covariances) at the very end.
  - banded matvecs become elementwise mul + strided reduce over (src, o).
  - the quadratic covariance terms use the factorization
        ncu = red(t11*(t11 vcu + t12 vcs) + t12*(t11 vcs + t12 vcl))
        ncs = red(t21*(t11 vcu + t12 vcs) + t22*(t11 vcs + t12 vcl))
        ncl = red(t21*(t21 vcu + t22 vcs) + t22*(t21 vcs + t22 vcl))
    which shares the inner sums (PQ, RS) and needs no explicit squares.
"""

import numpy as np

B_FULL = 8192
N_CORES = 8
BC = B_FULL // N_CORES          # 1024 rows per core
P = 128                         # partitions per tile
NT = BC // P                    # 8 tiles per core
LOD = 60
LSD = 120
AD = 10
NB = 15
BW = 3
NO = 2 * BW + 1                 # 7 diagonals
H = 60
SLAB = LOD * NO                 # 420
PADW = LOD + 2 * BW             # 66 padded columns per tile-slot

_BUILT = {}


def _build_nc():
    import concourse.bass as bass
    import concourse.bacc as bacc
    import concourse.mybir as mybir
    from concourse.tile import TileContext

    fp32 = mybir.dt.float32
    fp32r = mybir.dt.float32r
    AF = mybir.ActivationFunctionType
    OP = mybir.AluOpType

    nc = bacc.Bacc("TRN2", target_bir_lowering=False, debug=False)

    # ---- DRAM I/O ----
    pm_d = nc.declare_dram_parameter("pm", [BC, LSD], fp32, isOutput=False)
    cu_d = nc.declare_dram_parameter("cu", [BC, LOD], fp32, isOutput=False)
    cs_d = nc.declare_dram_parameter("cs", [BC, LOD], fp32, isOutput=False)
    cl_d = nc.declare_dram_parameter("cl", [BC, LOD], fp32, isOutput=False)
    act_d = nc.declare_dram_parameter("act", [BC, AD], fp32, isOutput=False)
    d1e_d = nc.declare_dram_parameter("d1e", [NB, 2 * SLAB], fp32, isOutput=False)
    d2e_d = nc.declare_dram_parameter("d2e", [NB, 2 * SLAB], fp32, isOutput=False)
    wcoefT_d = nc.declare_dram_parameter("wcoefT", [LSD, NB], fp32, isOutput=False)
    bcoef_d = nc.declare_dram_parameter("bcoef", [1, NB], fp32, isOutput=False)
    wc1T_d = nc.declare_dram_parameter("wc1T", [AD, H], fp32, isOutput=False)
    bc1_d = nc.declare_dram_parameter("bc1", [1, H], fp32, isOutput=False)
    wc2T_d = nc.declare_dram_parameter("wc2T", [H, LSD], fp32, isOutput=False)
    bc2_d = nc.declare_dram_parameter("bc2", [1, LSD], fp32, isOutput=False)
    tc_d = nc.declare_dram_parameter("tc", [1, 3 * LOD], fp32, isOutput=False)
    ident_d = nc.declare_dram_parameter("ident", [P, P], fp32, isOutput=False)

    nm_d = nc.declare_dram_parameter("next_mean", [BC, LSD], fp32, isOutput=True)
    ncu_d = nc.declare_dram_parameter("ncu", [BC, LOD], fp32, isOutput=True)
    ncl_d = nc.declare_dram_parameter("ncl", [BC, LOD], fp32, isOutput=True)
    ncs_d = nc.declare_dram_parameter("ncs", [BC, LOD], fp32, isOutput=True)

    def win_ap(tile_ap, col0):
        """Sliding-window read AP [P, 60, 7]: col = col0 + i + o."""
        base = tile_ap[:, col0:col0 + 1]
        part = base.ap[0]
        return bass.AP(
            tensor=base.tensor, offset=base.offset,
            ap=[[part[0], part[1]], [1, LOD], [1, NO]],
        )

    with TileContext(nc) as tc:
        with (
            tc.tile_pool(name="const", bufs=1) as cpool,
            tc.tile_pool(name="io", bufs=1) as iopool,
            tc.tile_pool(name="work", bufs=2) as wpool,
            tc.tile_pool(name="wsmall", bufs=4) as wsm,
            tc.tile_pool(name="psA", bufs=3, space="PSUM") as psA,
            tc.tile_pool(name="psmix", bufs=1, space="PSUM") as psmix,
            tc.tile_pool(name="psctl", bufs=1, space="PSUM") as psctl,
        ):
            # ---------- constants ----------
            d1e = cpool.tile([NB, 2 * SLAB], fp32, tag="d1e")
            d2e = cpool.tile([NB, 2 * SLAB], fp32, tag="d2e")
            wcoefT = cpool.tile([LSD, NB], fp32, tag="wcoefT")
            bcoef = cpool.tile([1, NB], fp32, tag="bcoef")
            wc1T = cpool.tile([AD, H], fp32, tag="wc1T")
            bc1 = cpool.tile([1, H], fp32, tag="bc1")
            wc2T = cpool.tile([H, LSD], fp32, tag="wc2T")
            bc2 = cpool.tile([1, LSD], fp32, tag="bc2")
            tcrow = cpool.tile([1, 3 * LOD], fp32, tag="tcrow")
            ident = cpool.tile([P, P], fp32, tag="ident")
            ones = cpool.tile([NB, P], fp32, tag="ones")
            tcbc = cpool.tile([P, 3 * LOD], fp32, tag="tcbc")

            nc.sync.dma_start(out=d1e[:], in_=d1e_d[:])
            nc.sync.dma_start(out=d2e[:], in_=d2e_d[:])
            nc.sync.dma_start(out=wcoefT[:], in_=wcoefT_d[:])
            nc.sync.dma_start(out=bcoef[:], in_=bcoef_d[:])
            nc.sync.dma_start(out=wc1T[:], in_=wc1T_d[:])
            nc.sync.dma_start(out=bc1[:], in_=bc1_d[:])
            nc.sync.dma_start(out=wc2T[:], in_=wc2T_d[:])
            nc.sync.dma_start(out=bc2[:], in_=bc2_d[:])
            nc.sync.dma_start(out=tcrow[:], in_=tc_d[:])
            nc.sync.dma_start(out=ident[:], in_=ident_d[:])
            nc.gpsimd.memset(ones[:], 1.0)

            # broadcast trans_cov to all partitions: [P, 120] = ones_col @ tc_row
            ps_tc = psA.tile([P, 3 * LOD], fp32, tag="psmall")
            nc.tensor.matmul(ps_tc[:], ones[0:1, :], tcrow[:])
            nc.scalar.copy(tcbc[:], ps_tc[:])

            # ---------- big input buffers ----------
            pm_all = iopool.tile([P, NT, LSD], fp32, tag="pm_all")
            act_all = iopool.tile([P, NT, AD], fp32, tag="act_all")
            mu_pad = iopool.tile([P, NT * PADW], fp32, tag="mu_pad")
            ml_pad = iopool.tile([P, NT * PADW], fp32, tag="ml_pad")
            cu_pad = iopool.tile([P, NT * PADW], fp32, tag="cu_pad")
            cs_pad = iopool.tile([P, NT * PADW], fp32, tag="cs_pad")
            cl_pad = iopool.tile([P, NT * PADW], fp32, tag="cl_pad")

            for buf in (mu_pad, ml_pad, cu_pad, cs_pad, cl_pad):
                nc.gpsimd.memset(buf[:], 0.0)

            nc.sync.dma_start(
                out=pm_all[:], in_=pm_d.rearrange("(t p) c -> p t c", p=P))
            nc.sync.dma_start(
                out=act_all[:], in_=act_d.rearrange("(t p) c -> p t c", p=P))

            def pad_view(buf):
                # [P, NT, 60] view of the data columns inside [P, NT*66]
                v = buf.rearrange("p (t c) -> p t c", c=PADW)
                return v[:, :, BW:BW + LOD]

            nc.sync.dma_start(
                out=pad_view(mu_pad),
                in_=pm_d[:, 0:LOD].rearrange("(t p) c -> p t c", p=P))
            nc.sync.dma_start(
                out=pad_view(ml_pad),
                in_=pm_d[:, LOD:LSD].rearrange("(t p) c -> p t c", p=P))
            nc.sync.dma_start(
                out=pad_view(cu_pad), in_=cu_d.rearrange("(t p) c -> p t c", p=P))
            nc.sync.dma_start(
                out=pad_view(cs_pad), in_=cs_d.rearrange("(t p) c -> p t c", p=P))
            nc.sync.dma_start(
                out=pad_view(cl_pad), in_=cl_d.rearrange("(t p) c -> p t c", p=P))

            # ---------- big output buffers ----------
            nm_all = iopool.tile([P, NT, LSD], fp32, tag="nm_all")
            cov_all = iopool.tile([P, NT, 3 * LOD], fp32, tag="cov_all")

            # ---------- per-tile compute ----------
            for t in range(NT):
                # --- softmax coefficient path ---
                ps_pmT = psA.tile([LSD, P], fp32, tag="psmall")
                nc.tensor.transpose(ps_pmT[:], pm_all[:, t, :], ident[:])
                pmT = wsm.tile([LSD, P], fp32, tag="pmT")
                nc.scalar.copy(pmT[:], ps_pmT[:])

                ps_log = psA.tile([NB, P], fp32, tag="psmall")
                nc.tensor.matmul(ps_log[:], wcoefT[:], pmT[:],
                                 start=True, stop=False)
                nc.tensor.matmul(ps_log[:], bcoef[:], ones[0:1, :],
                                 start=False, stop=True)
                expT = wsm.tile([NB, P], fp32, tag="expT")
                nc.scalar.activation(expT[:], ps_log[:], AF.Exp)

                ps_sbm = psA.tile([P, 1], fp32, tag="psmall")
                nc.tensor.matmul(ps_sbm[:], expT[:], ones[0:NB, 0:1])
                ssb = wsm.tile([P, 3], fp32, tag="ssb")
                nc.vector.tensor_copy(ssb[:, 0:1], ps_sbm[:])
                rinv = ssb[:, 1:2]
                rinv2 = ssb[:, 2:3]
                nc.vector.reciprocal(rinv, ssb[:, 0:1])
                nc.vector.tensor_tensor(rinv2, rinv, rinv, op=OP.mult)

                # --- mixes (raw = S * t_diag) -> SBUF [t11|t12|t21|t22] ---
                ps_mix1 = psmix.tile([P, 2 * SLAB], fp32, tag="mix1")
                ps_mix2 = psmix.tile([P, 2 * SLAB], fp32, tag="mix2")
                for ps, de in ((ps_mix1, d1e), (ps_mix2, d2e)):
                    nc.tensor.matmul(ps[:, 0:512], expT[:],
                                     de[:, 0:512], start=True, stop=True)
                    nc.tensor.matmul(ps[:, 512:840], expT[:],
                                     de[:, 512:840], start=True, stop=True)
                mix = wpool.tile([P, 4 * SLAB], fp32, tag="mix")
                nc.scalar.copy(mix[:, 0:840], ps_mix1[:])
                nc.scalar.copy(mix[:, 840:1680], ps_mix2[:])
                # identity contribution on center diags of t11 / t22 slabs
                # (ACT Identity with per-partition bias = +S)
                for c0 in (BW, 3 * SLAB + BW):
                    sl = mix[:, c0:c0 + SLAB - NO + 1:NO]
                    nc.scalar.activation(sl, sl, AF.Identity,
                                         bias=ssb[:, 0:1])

                # --- v_exp sliding windows [P, 5*420]: cu cs cl mu ml ---
                vex = wpool.tile([P, 5 * SLAB], fp32, tag="vex")
                c = t * PADW
                nc.scalar.copy(vex[:, 0 * SLAB:1 * SLAB], win_ap(cu_pad, c))
                nc.scalar.copy(vex[:, 1 * SLAB:2 * SLAB], win_ap(cs_pad, c))
                nc.scalar.copy(vex[:, 2 * SLAB:3 * SLAB], win_ap(cl_pad, c))
                nc.scalar.copy(vex[:, 3 * SLAB:4 * SLAB], win_ap(mu_pad, c))
                nc.scalar.copy(vex[:, 4 * SLAB:5 * SLAB], win_ap(ml_pad, c))

                # --- control net ---
                ps_actT = psA.tile([AD, P], fp32, tag="psmall")
                nc.tensor.transpose(ps_actT[:], act_all[:, t, :], ident[:])
                actT = wsm.tile([AD, P], fp32, tag="actT")
                nc.scalar.copy(actT[:], ps_actT[:])

                ps_h = psA.tile([P, H], fp32, tag="psmall")
                nc.tensor.matmul(ps_h[:], actT[:], wc1T[:], start=True, stop=False)
                nc.tensor.matmul(ps_h[:], ones[0:1, :], bc1[:],
                                 start=False, stop=True)
                hs = wsm.tile([P, H], fp32, tag="hs")
                nc.scalar.activation(hs[:], ps_h[:], AF.Relu)

                ps_hT = psA.tile([H, P], fp32, tag="psmall")
                nc.tensor.transpose(ps_hT[:], hs[:], ident[:])
                hT = wsm.tile([H, P], fp32, tag="hT")
                nc.scalar.copy(hT[:], ps_hT[:])

                ps_ctrl = psctl.tile([P, LSD], fp32, tag="ps_ctrl")
                nc.tensor.matmul(ps_ctrl[:], hT[:], wc2T[:], start=True, stop=False)
                nc.tensor.matmul(ps_ctrl[:], ones[0:1, :], bc2[:],
                                 start=False, stop=True)

                # --- quadratic tmps: tq = [T1|T2|T3|T4], Tj = t_j (x) v-window ---
                def rep_mix(off):
                    # [P, 2(j), 2(rep), 420]: t_{off}, t_{off+1} each repeated 2x
                    base = mix[:, off:off + 840]
                    p0 = base.ap[0]
                    return bass.AP(tensor=base.tensor, offset=base.offset,
                                   ap=[[p0[0], p0[1]], [SLAB, 2], [0, 2], [1, SLAB]])

                def v_pair(off):
                    # [P, 2(j), 2(rep-as-contig), 420]: [v_a|v_b] then [v_b|v_c]
                    base = vex[:, off:off + 1]
                    p0 = base.ap[0]
                    return bass.AP(tensor=base.tensor, offset=base.offset,
                                   ap=[[p0[0], p0[1]], [SLAB, 2], [SLAB, 2], [1, SLAB]])

                def blk4(tile, off, s0, c0, s1, c1, inner):
                    base = tile[:, off:off + 1]
                    p0 = base.ap[0]
                    return bass.AP(tensor=base.tensor, offset=base.offset,
                                   ap=[[p0[0], p0[1]], [s0, c0], [s1, c1], [1, inner]])

                tq = wpool.tile([P, 8 * SLAB], fp32, tag="tq")
                # T1|T2 = [t11,t12] (x) [[vcu|vcs],[vcs|vcl]]
                nc.vector.tensor_tensor(
                    blk4(tq, 0, 840, 2, 420, 2, SLAB), rep_mix(0), v_pair(0),
                    op=OP.mult)
                # T3|T4 = [t21,t22] (x) same windows
                nc.gpsimd.tensor_tensor(
                    blk4(tq, 1680, 840, 2, 420, 2, SLAB), rep_mix(840), v_pair(0),
                    op=OP.mult)

                # PQ|RS = [T1+T2 | T3+T4]
                pqrs = wpool.tile([P, 4 * SLAB], fp32, tag="pqrs")
                nc.gpsimd.tensor_tensor(
                    blk4(pqrs, 0, 840, 2, 420, 2, SLAB),
                    blk4(tq, 0, 1680, 2, 420, 2, SLAB),
                    blk4(tq, 840, 1680, 2, 420, 2, SLAB), op=OP.add)

                # --- g tensors ---
                g = wpool.tile([P, 6 * SLAB], fp32, tag="g")
                # [g_ncu | g_ncs] = [t11|t12|t21|t22] (x) [PQ|PQ]
                nc.vector.tensor_tensor(
                    blk4(g, 0, 840, 2, 420, 2, SLAB),
                    blk4(mix, 0, 840, 2, 420, 2, SLAB),
                    blk4(pqrs, 0, 0, 2, 420, 2, SLAB), op=OP.mult)
                # g_ncl = [t21|t22] (x) RS
                nc.gpsimd.tensor_tensor(g[:, 1680:2520], mix[:, 840:1680],
                                        pqrs[:, 840:1680], op=OP.mult)

                # g_mean = [t11|t12|t21|t22] (x) [vmu|vml|vmu|vml]
                gm = wpool.tile([P, 4 * SLAB], fp32, tag="gm")
                nc.vector.tensor_tensor(
                    blk4(gm, 0, 840, 2, 420, 2, SLAB),
                    blk4(mix, 0, 840, 2, 420, 2, SLAB),
                    blk4(vex, 3 * SLAB, 0, 2, 420, 2, SLAB), op=OP.mult)

                # --- reduce over (src, o) keeping (y, i) ---
                red = wsm.tile([P, 5 * LOD], fp32, tag="red")
                # cov: [ncu|ncs|ncl] ; mean: [nmu|nml]
                def reduce_g(dst, src_ap):
                    r = src_ap.rearrange("p (s i o) -> p i s o",
                                         s=2, i=LOD, o=NO)
                    nc.vector.tensor_reduce(dst, r, axis=mybir.AxisListType.XY,
                                            op=OP.add)

                reduce_g(red[:, 0:60], g[:, 0:840])           # ncu
                reduce_g(red[:, 60:120], g[:, 840:1680])      # ncs
                reduce_g(red[:, 120:180], g[:, 1680:2520])    # ncl
                reduce_g(red[:, 180:240], gm[:, 0:840])       # nmu
                reduce_g(red[:, 240:300], gm[:, 840:1680])    # nml

                # --- finals ---
                nc.vector.scalar_tensor_tensor(
                    nm_all[:, t, :], red[:, 180:300], rinv, ps_ctrl[:],
                    op0=OP.mult, op1=OP.add)
                nc.vector.scalar_tensor_tensor(
                    cov_all[:, t, :], red[:, 0:180], rinv2, tcbc[:],
                    op0=OP.mult, op1=OP.add)

            # ---------- stores ----------
            nc.sync.dma_start(
                out=nm_d.rearrange("(t p) c -> p t c", p=P), in_=nm_all[:])
            cv = cov_all.rearrange("p t c -> p t c") if False else cov_all
            nc.sync.dma_start(
                out=ncu_d.rearrange("(t p) c -> p t c", p=P),
                in_=cov_all[:, :, 0:LOD])
            nc.sync.dma_start(
                out=ncs_d.rearrange("(t p) c -> p t c", p=P),
                in_=cov_all[:, :, LOD:2 * LOD])
            nc.sync.dma_start(
                out=ncl_d.rearrange("(t p) c -> p t c", p=P),
                in_=cov_all[:, :, 2 * LOD:3 * LOD])

    nc.compile()
    return nc


def _prep_consts(inputs):
    mask = (np.abs(np.arange(LOD)[:, None] - np.arange(LOD)[None, :]) <= BW)
    ii, oo = np.meshgrid(np.arange(LOD), np.arange(NO), indexing="ij")
    jj = ii + oo - BW
    valid = (jj >= 0) & (jj < LOD)
    jj_c = np.clip(jj, 0, LOD - 1)

    def diag_slab(tm):
        A = np.asarray(tm, np.float32) * mask
        Dm = A[:, ii, jj_c] * valid          # [15, 60, 7]
        return np.ascontiguousarray(Dm.reshape(NB, SLAB), np.float32)

    d1e = np.concatenate([diag_slab(inputs["tm11"]), diag_slab(inputs["tm12"])], 1)
    d2e = np.concatenate([diag_slab(inputs["tm21"]), diag_slab(inputs["tm22"])], 1)
    ln = np.asarray(inputs["log_noise"], np.float32)
    tcv = np.where(ln < 0.0, np.exp(ln), ln + 1.0).reshape(LSD)
    tc = np.concatenate([tcv[:LOD], np.zeros(LOD, np.float32),
                         tcv[LOD:]]).reshape(1, 3 * LOD).astype(np.float32)
    consts = {
        "d1e": d1e,
        "d2e": d2e,
        "wcoefT": np.ascontiguousarray(np.asarray(inputs["w_coef"], np.float32).T),
        "bcoef": np.asarray(inputs["b_coef"], np.float32).reshape(1, NB),
        "wc1T": np.ascontiguousarray(np.asarray(inputs["w_c1"], np.float32).T),
        "bc1": np.asarray(inputs["b_c1"], np.float32).reshape(1, H),
        "wc2T": np.ascontiguousarray(np.asarray(inputs["w_c2"], np.float32).T),
        "bc2": np.asarray(inputs["b_c2"], np.float32).reshape(1, LSD),
        "tc": np.ascontiguousarray(tc, np.float32),
        "ident": np.eye(P, dtype=np.float32),
    }
    return consts


def _run(inputs, **kw):
    from concourse.bass_utils import run_bass_kernel_spmd

    if "nc" not in _BUILT:
        _BUILT["nc"] = _build_nc()
    nc = _BUILT["nc"]

    consts = _prep_consts(inputs)
    pm = np.ascontiguousarray(np.asarray(inputs["post_mean"], np.float32))
    cu = np.ascontiguousarray(np.asarray(inputs["cu"], np.float32))
    cs = np.ascontiguousarray(np.asarray(inputs["cs"], np.float32))
    cl = np.ascontiguousarray(np.asarray(inputs["cl"], np.float32))
    act = np.ascontiguousarray(np.asarray(inputs["action"], np.float32))

    in_maps = []
    for c in range(N_CORES):
        sl = slice(c * BC, (c + 1) * BC)
        m = {"pm": pm[sl], "cu": cu[sl], "cs": cs[sl], "cl": cl[sl],
             "act": act[sl]}
        m.update(consts)
        in_maps.append(m)

    return run_bass_kernel_spmd(nc, in_maps, core_ids=list(range(N_CORES)), **kw)


def kernel(**inputs):
    res = _run(inputs)
    nm = np.concatenate([r["next_mean"] for r in res.results], 0)
    ncu = np.concatenate([r["ncu"] for r in res.results], 0)
    ncl = np.concatenate([r["ncl"] for r in res.results], 0)
    ncs = np.concatenate([r["ncs"] for r in res.results], 0)
    return nm, ncu, ncl, ncs


def run_traced(inputs, tmpdir=None):
    res = _run(inputs, trace=True, tmpdir=tmpdir)
    return res.exec_time_ns
